# revision 72
# baseline (speedup 1.0000x reference)
"""Trainium2 Bass kernel for nn_Based_40630390620259 (sparse_attention).

Architecture ("Based"-style): linear (Taylor feature-map) attention +
windowed softmax attention, 16 heads, S=2048, D=1024.

Math identities (verified against the reference):
  - Taylor feature map inner product collapses:
        qf.kf = 1 + (q.k)/4 + (q.k)^2/32 = 0.5 + 0.5*(1 + q.k/4)^2
    so the 273-dim feature space is never materialized. With Wq,Wk scaled
    by 0.5 on the host and a constant ones-row appended to q/k (K=17
    matmul), the PE produces m'' = 1 + q.k/4 directly; sq = m''^2 on ACT.
  - The 0.5 factor is folded into the V projection weights; the +0.5
    constant term contributes a causal cumulative sum CUM of the
    (0.5-scaled) v rows (16 N=128 matmuls against a triangular ones block
    + a recursive per-partition scalar-add). CUM row 64 equals 0.5*(s+1).
  - win path: scores^T computed as [t,s] tiles; softmax denominator via a
    ones-column in V'; division deferred: reciprocal rows live on
    partition 0 and are broadcast along partitions with gpsimd
    partition_broadcast (no DMA hops anywhere in the tails).

Sharding: tensor-parallel over heads, 2 heads per core, 8 cores. Each core
produces a partial [S, D] output (its heads' contribution); the host sums.

Scheduling notes (cost-model driven):
  - Single unified emission stream: projection work for s-blocks 2,3 is
    interleaved as PE padding inside the attention phases of s-blocks 0,1
    so the ACT engine (Square/Exp, the local phase-2 bottleneck) is never
    the head-of-line blocker for the in-order PE queue.
  - One unified PSUM pool (8 banks exactly): pp[2x1] proj/finals,
    psA[2x2] score tiles, acc[2x1] attention accumulators.
  - Inputs arrive via 10 large DMAs (HWDGE is a serial 625ns/DMA device),
    ordered so the first projection starts at ~4.7us.
  - ACT runs ONLY Square/Exp; every staging copy goes to DVE; masks and
    broadcasts to Pool.

Self-contained: only imports concourse/* from the environment.
"""

import numpy as np
import ml_dtypes

S = 2048
D = 1024
H = 16
FD = 16
HD = 64
W = 256
EPS = 1e-9
NCORES = 8

BF = ml_dtypes.bfloat16

_CACHE = {}


def _build_nc(dbg=False):
    import concourse.bass as bass
    import concourse.mybir as mybir
    import concourse.tile as tile
    from concourse import bacc
    from concourse.bass import ts

    f32 = mybir.dt.float32
    bf16 = mybir.dt.bfloat16
    MULT = mybir.AluOpType.mult
    ADD = mybir.AluOpType.add
    Exp = mybir.ActivationFunctionType.Exp
    Square = mybir.ActivationFunctionType.Square

    nc = bacc.Bacc("TRN2", target_bir_lowering=False)

    ht_d = nc.dram_tensor("ht", [D, S], bf16, kind="ExternalInput")
    wqk_d = nc.dram_tensor("wqk", [D, 384], bf16, kind="ExternalInput")
    wv_d = nc.dram_tensor("wv", [D, 256], bf16, kind="ExternalInput")
    wo_d = nc.dram_tensor("wo", [256, D], bf16, kind="ExternalInput")
    msk_d = nc.dram_tensor("msk", [128, 256], bf16, kind="ExternalInput")
    oh_d = nc.dram_tensor("oh", [128, 1], f32, kind="ExternalInput")
    out_d = nc.dram_tensor("out", [S, D], bf16, kind="ExternalOutput")

    NJ = 4          # number of 512-wide s blocks
    SB = 512        # s block width
    NT = 16         # number of 128-wide t chunks

    with tile.TileContext(nc) as tc:
        with (
            tc.tile_pool(name="sb", bufs=1) as sb,
            tc.tile_pool(name="sqp", bufs=12) as sqp,
            tc.tile_pool(name="exp", bufs=12) as exq,
            tc.tile_pool(name="stg", bufs=6) as stg,
            tc.tile_pool(name="bct", bufs=3) as bct,
            tc.tile_pool(name="rcp", bufs=3) as rcp,
            tc.tile_pool(name="ps", bufs=2, space="PSUM") as ps,
        ):
            # ---------------- persistent SBUF tiles ----------------
            ht_sb = sb.tile([128, 8, S], bf16, name="ht_sb")
            wqk_sb = sb.tile([128, 8, 384], bf16, name="wqk_sb")
            wv_sb = sb.tile([128, 8, 256], bf16, name="wv_sb")
            wo_sb = sb.tile([128, 2, 1024], bf16, name="wo_sb")
            msk_sb = sb.tile([128, 256], bf16, name="msk_sb")
            # qkg: lin q/k with ones row; h0 rows 0:17, h1 rows 32:49;
            # free index 0 = q, 1 = k
            qkg_sb = sb.tile([128, 2, S], bf16, name="qkg_sb")
            qw_sb = sb.tile([128, S], bf16, name="qw_sb")
            kw_sb = sb.tile([128, S], bf16, name="kw_sb")
            v4_sb = sb.tile([128, NT, 260], bf16, name="v4_sb")
            cum_sb = [sb.tile([65, S], f32, name=f"cum{h}_sb") for h in range(2)]
            ul_sb = [sb.tile([65, S], f32, name=f"ul{h}_sb") for h in range(2)]
            # scl: [lin(0:64)|win(64:128) rows, h, s]
            scl_sb = sb.tile([128, 2, S], bf16, name="scl_sb")
            oh_sb = sb.tile([128, 1], f32, name="oh_sb")

            v4r = v4_sb.rearrange("p s (g x) -> p s g x", x=65)

            # ---------------- init constants ----------------
            nc.gpsimd.memset(v4r[:, :, 0:2, 64], 0.5)
            nc.gpsimd.memset(v4r[:, :, 2:4, 64], 1.0)

            # ---------------- input DMAs (few, big, ordered) --------------
            htr = ht_d.rearrange("(k p) s -> p k s", p=128)
            wqkr = wqk_d.rearrange("(k p) c -> p k c", p=128)
            wvr = wv_d.rearrange("(k p) c -> p k c", p=128)
            wor = wo_d.rearrange("(k p) c -> p k c", p=128)
            for kk in range(4):
                k0, k1 = 2 * kk, 2 * kk + 2
                nc.sync.dma_start(out=wqk_sb[:, k0:k1, :], in_=wqkr[:, k0:k1, :])
                nc.sync.dma_start(
                    out=ht_sb[:, k0:k1, 0:SB], in_=htr[:, k0:k1, 0:SB]
                )
            nc.sync.dma_start(out=wv_sb[:, :, :], in_=wvr[:, :, :])
            nc.sync.dma_start(out=oh_sb[:, :], in_=oh_d[:, :])
            nc.sync.dma_start(out=msk_sb[:, :], in_=msk_d[:, :])
            for j in range(1, 4):
                nc.sync.dma_start(
                    out=ht_sb[:, :, ts(j, SB)], in_=htr[:, :, ts(j, SB)]
                )
            nc.sync.dma_start(out=wo_sb[:, :, :], in_=wor[:, :, :])

            # =============== phase-1 unit generators ======================
            # Each unit is a closure doing ~1-3.5us of PE work; units for
            # j-blocks 2,3 are sprinkled into the attention phases as PE
            # padding (ACT never blocks the PE queue head).

            def p1a_j0_unit():
                # chunk-major across all 3 blocks: consumes each arriving
                # ht chunk-pair at once so the PE never waits the next DMA
                def run():
                    js = ts(0, SB)
                    pps = [
                        ps.tile([128, SB], f32, name=f"pp0_{blk}", tag="pp")
                        for blk in range(2)
                    ] + [ps.tile([128, SB], f32, name="pp0_2", tag="psA")]
                    for k in range(8):
                        for blk in range(3):
                            nc.tensor.matmul(
                                pps[blk][:, :],
                                lhsT=wqk_sb[:, k, ts(blk, 128)],
                                rhs=ht_sb[:, k, js],
                                start=(k == 0),
                                stop=(k == 7),
                            )
                    nc.vector.tensor_copy(out=qw_sb[:, js], in_=pps[0][:, :])
                    nc.vector.tensor_copy(out=kw_sb[:, js], in_=pps[1][:, :])
                    for qk in range(2):
                        nc.vector.tensor_scalar(
                            out=qkg_sb[0:49, qk, js],
                            in0=pps[2][64 * qk : 64 * qk + 49, :],
                            scalar1=oh_sb[64 * qk : 64 * qk + 49, 0:1],
                            scalar2=None,
                            op0=ADD,
                        )
                return run

            def p1a_unit(j, blk):
                def run():
                    js = ts(j, SB)
                    pp = ps.tile(
                        [128, SB], f32, name=f"pp{j}_{blk}", tag="pp"
                    )
                    for k in range(8):
                        nc.tensor.matmul(
                            pp[:, :],
                            lhsT=wqk_sb[:, k, ts(blk, 128)],
                            rhs=ht_sb[:, k, js],
                            start=(k == 0),
                            stop=(k == 7),
                        )
                    if blk == 0:
                        nc.vector.tensor_copy(out=qw_sb[:, js], in_=pp[:, :])
                    elif blk == 1:
                        nc.vector.tensor_copy(out=kw_sb[:, js], in_=pp[:, :])
                    else:
                        # q rows 0:49 and k rows 64:113 with fused ones rows
                        # (+1 at local rows 16/48 via the onehot column)
                        for qk in range(2):
                            nc.vector.tensor_scalar(
                                out=qkg_sb[0:49, qk, js],
                                in0=pp[64 * qk : 64 * qk + 49, :],
                                scalar1=oh_sb[64 * qk : 64 * qk + 49, 0:1],
                                scalar2=None,
                                op0=ADD,
                            )
                return run

            def p1b_unit(j, spi):
                def run():
                    sp_ = 2 * j + spi
                    st0 = 2 * sp_
                    pv = ps.tile([128, 512], f32, name=f"pv{sp_}", tag="pp")
                    for sh in range(2):
                        for k in range(8):
                            nc.tensor.matmul(
                                pv[:, sh * 256 : sh * 256 + 256],
                                lhsT=ht_sb[:, k, ts(st0 + sh, 128)],
                                rhs=wv_sb[:, k, :],
                                start=(k == 0),
                                stop=(k == 7),
                            )
                    nc.vector.tensor_copy(
                        out=v4r[:, st0 : st0 + 2, :, 0:64],
                        in_=pv[:, 0:512].rearrange(
                            "p (s g x) -> p s g x", s=2, x=64
                        ),
                    )
                return run

            def cum_unit(j):
                # All 8 intra-chunk matmuls first (independent, PE streams
                # through), then the chained per-chunk adds run DVE-only so
                # the serial cumsum never blocks the PE queue head.
                def run():
                    icp = ps.tile(
                        [65, 2, 4, 128], f32, name=f"icp{j}", tag="psA"
                    )
                    for sjl in range(4):
                        sj = 4 * j + sjl
                        for h in range(2):
                            nc.tensor.matmul(
                                icp[:, h, sjl, :],
                                lhsT=v4_sb[:, sj, 65 * h : 65 * h + 65],
                                rhs=msk_sb[:, 0:128],
                                start=True,
                                stop=True,
                            )
                    for sjl in range(4):
                        sj = 4 * j + sjl
                        for h in range(2):
                            nc.vector.tensor_scalar(
                                out=cum_sb[h][:, ts(sj, 128)],
                                in0=icp[:, h, sjl, :],
                                scalar1=(
                                    0.0
                                    if sj == 0
                                    else cum_sb[h][:, sj * 128 - 1 : sj * 128]
                                ),
                                scalar2=None,
                                op0=ADD,
                            )
                return run

            def p1_units(j):
                a_units = (
                    [p1a_j0_unit()]
                    if j == 0
                    else [p1a_unit(j, blk) for blk in range(3)]
                )
                return (
                    a_units
                    + [p1b_unit(j, spi) for spi in range(2)]
                    + [cum_unit(j)]
                )

            # =============== phase-2 machinery ============================
            def lin_tail(j, st_):
                """ul add + recip + broadcast + lin scales, inline."""
                js = ts(j, SB)
                rec = st_["rec"]
                for h in range(2):
                    nc.vector.tensor_tensor(
                        out=ul_sb[h][:, js],
                        in0=st_["qkv"][h][0:65, :],
                        in1=cum_sb[h][:, js],
                        op=ADD,
                    )
                with nc.allow_low_precision("bf16 reciprocal rows"):
                    for h in range(2):
                        nc.vector.reciprocal(
                            out=rec[0:1, 0, h, :],
                            in_=ul_sb[h][64:65, js],
                        )
                bcl = bct.tile([64, 2, SB], bf16, name=f"bcl{j}", tag="bcl")
                nc.gpsimd.partition_broadcast(bcl[:, :, :], rec[0:1, 0, :, :])
                for h in range(2):
                    nc.vector.tensor_tensor(
                        out=scl_sb[0:64, h, js],
                        in0=ul_sb[h][0:64, js],
                        in1=bcl[:, h, :],
                        op=MULT,
                    )

            def win_tail(j, st_):
                """recip + broadcast + win scales; reads nt PSUM directly."""
                js = ts(j, SB)
                rec = st_["rec"]
                with nc.allow_low_precision("bf16 reciprocal rows"):
                    for h in range(2):
                        nc.vector.reciprocal(
                            out=rec[0:1, 1, h, :],
                            in_=st_["nt"][h][64:65, :],
                        )
                bcw = bct.tile([64, 2, SB], bf16, name=f"bcw{j}", tag="bcw")
                if j < 3:
                    nc.gpsimd.partition_broadcast(
                        bcw[:, :, :], rec[0:1, 1, :, :]
                    )
                    for h in range(2):
                        nc.vector.tensor_tensor(
                            out=scl_sb[64:128, h, js],
                            in0=st_["nt"][h][0:64, :],
                            in1=bcw[:, h, :],
                            op=MULT,
                        )
                else:
                    # last tail: per-(h, half) broadcasts + st-split scales
                    # so the final matmuls start as early as possible
                    for h in range(2):
                        for hf in range(2):
                            nc.gpsimd.partition_broadcast(
                                bcw[:, h, hf * 256 : hf * 256 + 256],
                                rec[0:1, 1, h, hf * 256 : hf * 256 + 256],
                            )
                    for sd in range(4):
                        for h in range(2):
                            nc.vector.tensor_tensor(
                                out=scl_sb[64:128, h, ts(4 * j + sd, 128)],
                                in0=st_["nt"][h][0:64, ts(sd, 128)],
                                in1=bcw[:, h, ts(sd, 128)],
                                op=MULT,
                            )

            def phase2_blocks(j):
                js = ts(j, SB)
                lim_l = 4 * j + 4
                lim_w = min(16, 4 * j + 6)
                st_ = {}

                def begin():
                    st_["qkv"] = [
                        ps.tile([65, SB], f32, name=f"qkv{h}", tag="acc")
                        for h in range(2)
                    ]
                    st_["rec"] = rcp.tile(
                        [1, 2, 2, SB], bf16, name=f"rec{j}", tag="rec"
                    )
                    st_["sq"] = {}

                def lin_blk(ti):
                    mp = ps.tile([128, 1024], f32, name="mp", tag="psA")
                    col0 = max(0, ti - 4 * j) * 128
                    for h, (p0, p1_) in enumerate(((0, 17), (32, 49))):
                        nc.tensor.matmul(
                            mp[:, h * SB + col0 : (h + 1) * SB],
                            lhsT=qkg_sb[p0:p1_, 1, ts(ti, 128)],
                            rhs=qkg_sb[p0:p1_, 0, j * SB + col0 : (j + 1) * SB],
                            start=True,
                            stop=True,
                        )
                    sq = sqp.tile([128, 1024], bf16, name="sq", tag="sq")
                    sqr = sq.rearrange("p (g x) -> p g x", x=SB)
                    mpr = mp.rearrange("p (g x) -> p g x", x=SB)
                    if ti >= lim_l - 2:
                        # split per head so the flush accs chain off their
                        # own half instead of the full-width activation
                        for h in range(2):
                            nc.scalar.activation(
                                sqr[:, h : h + 1, col0:SB],
                                mpr[:, h : h + 1, col0:SB],
                                Square,
                            )
                    else:
                        nc.scalar.activation(
                            sqr[:, :, col0:SB], mpr[:, :, col0:SB], Square
                        )
                    sd = ti - 4 * j
                    if 0 <= sd <= 3:
                        for h in range(2):
                            dsl = slice(h * SB + sd * 128, h * SB + (sd + 1) * 128)
                            nc.gpsimd.tensor_tensor(
                                out=sq[:, dsl], in0=sq[:, dsl],
                                in1=msk_sb[:, 0:128], op=MULT,
                            )
                    st_["sq"][ti] = sq

                def lin_acc(ti):
                    sqt = st_["sq"].pop(ti)
                    col0 = max(0, ti - 4 * j) * 128
                    for h in range(2):
                        nc.tensor.matmul(
                            st_["qkv"][h][:, col0:SB],
                            lhsT=v4_sb[:, ti, slice(65 * h, 65 * h + 65)],
                            rhs=sqt[:, h * SB + col0 : (h + 1) * SB],
                            start=(ti == 0),
                            stop=(ti == lim_l - 1),
                            skip_group_check=True,
                        )

                def win_begin():
                    st_["nt"] = [
                        ps.tile([65, SB], f32, name=f"nt{h}", tag="acc")
                        for h in range(2)
                    ]
                    st_["ex"] = {}

                def win_blk(ti):
                    sp = ps.tile([128, 1024], f32, name="sp", tag="psA")
                    col0 = max(0, ti - 2 - 4 * j) * 128
                    for h in range(2):
                        hsl = slice(64 * h, 64 * h + 64)
                        nc.tensor.matmul(
                            sp[:, h * SB + col0 : (h + 1) * SB],
                            lhsT=kw_sb[hsl, ts(ti, 128)],
                            rhs=qw_sb[hsl, j * SB + col0 : (j + 1) * SB],
                            start=True,
                            stop=True,
                        )
                    ex = exq.tile([128, 1024], bf16, name="ex", tag="ex")
                    exr = ex.rearrange("p (g x) -> p g x", x=SB)
                    spr = sp.rearrange("p (g x) -> p g x", x=SB)
                    if ti >= lim_w - 2:
                        for h in range(2):
                            nc.scalar.activation(
                                exr[:, h : h + 1, col0:SB],
                                spr[:, h : h + 1, col0:SB],
                                Exp,
                                scale=0.125,
                            )
                    else:
                        nc.scalar.activation(
                            exr[:, :, col0:SB], spr[:, :, col0:SB], Exp, scale=0.125
                        )
                    sd = ti - 2 - 4 * j
                    if 0 <= sd <= 3:
                        for h in range(2):
                            dsl = slice(h * SB + sd * 128, h * SB + (sd + 1) * 128)
                            nc.gpsimd.tensor_tensor(
                                out=ex[:, dsl], in0=ex[:, dsl],
                                in1=msk_sb[:, 128:256], op=MULT,
                            )
                    st_["ex"][ti] = ex

                def win_acc(ti):
                    ext = st_["ex"].pop(ti)
                    col0 = max(0, ti - 2 - 4 * j) * 128
                    for h in range(2):
                        nc.tensor.matmul(
                            st_["nt"][h][:, col0:SB],
                            lhsT=v4_sb[:, ti, slice(130 + 65 * h, 195 + 65 * h)],
                            rhs=ext[:, h * SB + col0 : (h + 1) * SB],
                            start=(ti == 0),
                            stop=(ti == lim_w - 1),
                            skip_group_check=True,
                        )

                # block list: lin chain then win chain, lag-2 accumulation
                LGL = min(4, lim_l - 1) if lim_l > 3 else 2
                blocks = [begin]
                for ti in range(lim_l):
                    blocks.append(lambda ti=ti: lin_blk(ti))
                    if ti >= LGL:
                        blocks.append(lambda ti=ti - LGL: lin_acc(ti))
                blocks.append("LIN_FLUSH")
                for ti in range(lim_l - LGL, lim_l):
                    blocks.append(lambda ti=ti: lin_acc(ti))
                blocks.append(lambda: lin_tail(j, st_))
                blocks.append("WIN_START")
                LGW = 5
                blocks.append(win_begin)
                for ti in range(lim_w):
                    blocks.append(lambda ti=ti: win_blk(ti))
                    if ti >= LGW:
                        blocks.append(lambda ti=ti - LGW: win_acc(ti))
                blocks.append("FLUSH_START")
                for ti in range(lim_w - LGW, lim_w):
                    blocks.append(lambda ti=ti: win_acc(ti))
                blocks.append(lambda: win_tail(j, st_))
                return blocks

            _fcp = [0]

            def final_pieces(j):
                """Output projection, split per (strip, half) into 8 pieces
                so padding spreads finer; copies alternate DVE/ACT."""
                pieces = []
                last = j == 3
                sos = {}

                def final_half(st, nb):
                    if nb == 0:
                        sos[st] = stg.tile([128, 1024], bf16, name="so", tag="so")
                    so = sos[st]
                    # in the last tail alternate between two PSUM rings
                    # (psA slots are free by then) for 4-deep buffering
                    tag = "psA" if (last and nb == 1) else "pp"
                    po = ps.tile([128, SB], f32, name="po", tag=tag)
                    for h in range(2):
                        nc.tensor.matmul(
                            po[:, :],
                            lhsT=scl_sb[:, h, ts(st, 128)],
                            rhs=wo_sb[:, h, ts(nb, SB)],
                            start=(h == 0),
                            stop=(h == 1),
                            skip_group_check=True,
                        )
                    # ACT copies only in the last tail (ACT idle there);
                    # mid-kernel they would queue behind Square/Exp and
                    # stall the PE through the PSUM-ring WAR.
                    _fcp[0] ^= 1
                    if last and _fcp[0]:
                        nc.scalar.copy(out=so[:, ts(nb, SB)], in_=po[:, :])
                    else:
                        nc.vector.tensor_copy(out=so[:, ts(nb, SB)], in_=po[:, :])
                    if nb == 1:
                        nc.sync.dma_start(out=out_d[ts(st, 128), :], in_=so[:, :])

                def final_full(st):
                    final_half(st, 0)
                    final_half(st, 1)

                for st in range(4 * j, 4 * j + 4):
                    for nb in range(2):
                        pieces.append(lambda st=st, nb=nb: final_half(st, nb))
                return pieces

            def run_phase(j, early):
                blocks = phase2_blocks(j)
                lstart = blocks.index("LIN_FLUSH")
                blocks.remove("LIN_FLUSH")
                wstart = blocks.index("WIN_START")
                blocks.remove("WIN_START")
                fstart = blocks.index("FLUSH_START")
                blocks.remove("FLUSH_START")
                pi = 0
                post = 2 if len(early) >= 6 else 0
                reserve = min(3, len(early) - post)
                nlin = max(1, (len(early) - reserve - post) // 3)
                nwin = len(early) - reserve - post - nlin
                lin_stride = max(1, (wstart - 1) // (nlin + 1))
                win_stride = max(1, (fstart - wstart) // (nwin + 1)) if nwin else 10**9
                for i, b in enumerate(blocks):
                    if i == fstart:
                        # fill the ACT-backlog stall at the accumulation
                        # flush with independent PE work
                        while pi < len(early) - post:
                            early[pi]()
                            pi += 1
                    b()
                    if i < wstart:
                        if i >= 2 and (i - 1) % lin_stride == 0 and pi < nlin:
                            early[pi]()
                            pi += 1
                    elif i < fstart:
                        if (i - wstart + 1) % win_stride == 0 and pi < nlin + nwin:
                            early[pi]()
                            pi += 1
                while pi < len(early):
                    early[pi]()
                    pi += 1

            # =============== unified emission ============================
            # phase 1 for j-blocks 0,1 plainly (DMA-paced anyway)
            for u in p1_units(0):
                u()
            for u in p1_units(1):
                u()
            # attention phases with projection leftovers + finals as padding
            u3 = p1_units(3)
            f0 = final_pieces(0)
            run_phase(0, p1_units(2) + u3[:2])
            run_phase(1, u3[2:5] + f0[:6])
            f1 = final_pieces(1)
            run_phase(2, u3[5:] + f0[6:] + f1[:3])
            run_phase(3, f1[3:] + final_pieces(2))
            for p in final_pieces(3):
                p()

    nc.compile()
    return nc


def _prep_inputs(inputs):
    """Host-side sharding/packing. Returns per-core input maps."""
    h = np.asarray(inputs["hidden_states"], np.float32).reshape(S, D)
    ht = np.ascontiguousarray(h.T).astype(BF)

    lin_Wq = np.asarray(inputs["lin_Wq"], np.float32)
    lin_Wk = np.asarray(inputs["lin_Wk"], np.float32)
    lin_Wv = np.asarray(inputs["lin_Wv"], np.float32)
    lin_Wo = np.asarray(inputs["lin_Wo"], np.float32)
    win_Wq = np.asarray(inputs["win_Wq"], np.float32)
    win_Wk = np.asarray(inputs["win_Wk"], np.float32)
    win_Wv = np.asarray(inputs["win_Wv"], np.float32)
    win_Wo = np.asarray(inputs["win_Wo"], np.float32)

    # constant mask tiles
    p = np.arange(128)[:, None]
    f = np.arange(128)[None, :]
    msk = np.zeros((128, 256), np.float32)
    msk[:, 0:128] = (p <= f)          # lin diag mask (t <= s)
    msk[:, 128:256] = (p < f)         # win partial mask (t < s)

    in_maps = []
    for c in range(NCORES):
        a, b = 2 * c, 2 * c + 1
        wqk = np.zeros((D, 384), np.float32)
        wqk[:, 0:64] = win_Wq[:, a * HD : (a + 1) * HD]
        wqk[:, 64:128] = win_Wq[:, b * HD : (b + 1) * HD]
        wqk[:, 128:192] = win_Wk[:, a * HD : (a + 1) * HD]
        wqk[:, 192:256] = win_Wk[:, b * HD : (b + 1) * HD]
        # lin q/k slots are 17 wide: 16 weight cols + a zero col that the
        # kernel turns into the ones row via a fused +1 copy
        wqk[:, 256:272] = lin_Wq[:, a * FD : (a + 1) * FD] * 0.5
        wqk[:, 288:304] = lin_Wq[:, b * FD : (b + 1) * FD] * 0.5
        wqk[:, 320:336] = lin_Wk[:, a * FD : (a + 1) * FD] * 0.5
        wqk[:, 352:368] = lin_Wk[:, b * FD : (b + 1) * FD] * 0.5
        wv = np.zeros((D, 256), np.float32)
        wv[:, 0:64] = lin_Wv[:, a * HD : (a + 1) * HD] * 0.5
        wv[:, 64:128] = lin_Wv[:, b * HD : (b + 1) * HD] * 0.5
        wv[:, 128:192] = win_Wv[:, a * HD : (a + 1) * HD]
        wv[:, 192:256] = win_Wv[:, b * HD : (b + 1) * HD]
        wo = np.zeros((256, D), np.float32)
        wo[0:64] = lin_Wo[a * HD : (a + 1) * HD]
        wo[64:128] = win_Wo[a * HD : (a + 1) * HD]
        wo[128:192] = lin_Wo[b * HD : (b + 1) * HD]
        wo[192:256] = win_Wo[b * HD : (b + 1) * HD]
        oh = np.zeros((128, 1), np.float32)
        oh[[16, 48, 80, 112]] = 1.0
        in_maps.append(
            {
                "ht": ht,
                "wqk": wqk.astype(BF),
                "wv": wv.astype(BF),
                "wo": wo.astype(BF),
                "msk": msk.astype(BF),
                "oh": oh,
            }
        )
    return in_maps


def kernel(**inputs) -> np.ndarray:
    from concourse.bass_utils import run_bass_kernel_spmd

    if "nc" not in _CACHE:
        _CACHE["nc"] = _build_nc()
    nc = _CACHE["nc"]
    in_maps = _prep_inputs(inputs)
    res = run_bass_kernel_spmd(nc, in_maps, core_ids=list(range(NCORES)))
    out = np.zeros((S, D), np.float32)
    for r in res.results:
        out += np.asarray(r["out"], np.float32)
    return out.reshape(1, S, D)


if __name__ == "__main__":
    nc = _build_nc()
    print("built ok")
